# revision 9
# baseline (speedup 1.0000x reference)
"""Debiased EMA (nn_DebiasedEMA) Trainium2 Bass kernel.

x: [B=32, T=4096, C=512] f32.
    y_t = a*y_{t-1} + (1-a)*x_t  (y_0 = x_0), a = f32(0.9)
    out_t = y_t / max(1 - a^(t+1), 1e-6)

Formulation (exact, no window truncation): split T into 128-row blocks in
the natural [T, C] layout.  For block i with local row m,

    y_{128i+m} = (C.T @ x_block_i)[m]  +  a^(m+1) * carry_{i-1}

where C[k, m] = (1-a)*a^(m-k) for m >= k (triangular) and carry_{i-1} =
y_{128i-1} is the last EMA row of the previous block.  The carry term is a
RANK-1 matmul: lhsT = v (v[m] = a^(m+1), [1, 128]), rhs = carry ([1, C]).

Each NeuronCore runs 4 independent sequences (batch sharding), giving 4
independent carry chains.  Their K=1 rank-1 matmuls are packed into the 4
PE row-groups via tile_position, so 4 of them cost about one matmul
N-stream instead of four.  PE work per block ~ 1 fp32 matmul + 1/4 packed
rank-1 instead of the 2 fp32 matmuls of the sliding-window formulation.

The debias divisor is exactly 1.0 in fp32 for t >= 165, so it is folded
into the block-0/1 C matrices (block 0 also carries the y_0 = x_0 initial
condition) and into the block-1 v vector; carries are extracted from the
debiased output rows with the inverse fold (only block 0's divisor differs
from 1.0 at row 127).

Carry extraction is a 2KB SBUF->SBUF DMA (output row 127 -> partition 32b
of a carry tile, so batch b's rank-1 sits in row-group b).

Sharding: batch-parallel, 4 sequences per NeuronCore, no communication.
"""

import sys

for _p in ("/opt/trn_rl_repo", "/opt/pypackages"):
    if _p not in sys.path:
        sys.path.insert(0, _p)

import numpy as np

import concourse.bacc as bacc
import concourse.mybir as mybir
from concourse import bass_utils
from concourse.tile import TileContext

B, T, C = 32, 4096, 512
NCORES = 8
BPC = B // NCORES        # sequences per core
L = 128                  # time-block length == partition count
ALPHA = 0.9
DENOM_MIN = 1e-6

F32 = mybir.dt.float32


def _alpha_denoms():
    a = float(np.float32(ALPHA))     # f32 value of clip(0.9) as f64
    omb = 1.0 - a                    # exact (Sterbenz), matches f32 1-a
    t = np.arange(2 * L, dtype=np.float64)
    d = np.maximum(1.0 - a ** (t + 1.0), DENOM_MIN)
    return a, omb, d


def _build_weights() -> np.ndarray:
    """Three 128x128 current-block lhsT matrices, packed [128, 384] f32.

    lhsT layout: [k (input time, partitions), m (output time, free)];
    matmul computes out[m, n] = sum_k lhsT[k, m] * x[k, n].
    Built in float64 from the f32-rounded alpha, rounded once to f32.
    Columns: [A0 (block 0, debias+init fold) | C1 (block 1, debias fold) |
    Cm (blocks >= 2)].
    """
    a, omb, d = _alpha_denoms()
    k = np.arange(L, dtype=np.float64)[:, None]   # input index
    m = np.arange(L, dtype=np.float64)[None, :]   # output index
    tri = (m - k) >= 0
    dec = np.where(tri, a ** np.where(tri, m - k, 0.0), 0.0)     # a^(m-k)
    x0col = (k == 0)
    # block 0: y_m = a^m x_0 + (1-a) sum_{s=1..m} a^(m-s) x_s, rows / d[m]
    A0 = np.where(tri, np.where(x0col, a**m, omb * dec), 0.0) / d[:L][None, :]
    # block 1: current-block part, rows / d[128+m]
    C1 = omb * dec / d[L:][None, :]
    # blocks >= 2 (debias == 1.0 exactly in f32)
    Cm = omb * dec
    w = np.concatenate([A0, C1, Cm], axis=1)
    return np.ascontiguousarray(w.astype(np.float32))


def _build_v() -> np.ndarray:
    """Rank-1 carry row-vectors, [128, 256] f32.

    Row 32b (for batch slot b) holds: cols 0:128 = v1 (block 1, includes
    the undo of block 0's row-127 debias and block 1's debias fold), cols
    128:256 = v (blocks >= 2, plain a^(m+1)).  Other rows are zero.
    """
    a, omb, d = _alpha_denoms()
    m = np.arange(L, dtype=np.float64)
    v1 = (a ** (m + 1.0)) * d[L - 1] / d[L:]
    v = a ** (m + 1.0)
    out = np.zeros((L, 2 * L), dtype=np.float64)
    for b in range(4):
        out[32 * b, 0:L] = v1
        out[32 * b, L:] = v
    return np.ascontiguousarray(out.astype(np.float32))


def build_program(bpc: int = BPC, t_len: int = T, chunk: int = 4):
    """One core's program: EMA over `bpc` independent [t_len, C] sequences,
    batch-interleaved so the rank-1 carry matmuls pack across row groups."""
    nblk = t_len // L
    nchunk = nblk // chunk
    assert nblk * L == t_len and nchunk * chunk == nblk
    assert bpc <= 4

    nc = bacc.Bacc("TRN2", target_bir_lowering=False, debug=False)
    x = nc.dram_tensor("x", [bpc * t_len, C], F32, kind="ExternalInput").ap()
    w = nc.dram_tensor("w", [L, 3 * L], F32, kind="ExternalInput").ap()
    v = nc.dram_tensor("v", [L, 2 * L], F32, kind="ExternalInput").ap()
    y = nc.dram_tensor("y", [bpc * t_len, C], F32, kind="ExternalOutput").ap()

    with TileContext(nc) as tc:
        with (
            tc.tile_pool(name="wpool", bufs=1) as wpool,
            tc.tile_pool(name="xpool", bufs=2 * bpc) as xpool,
            tc.tile_pool(name="ypool", bufs=2 * bpc) as ypool,
            tc.tile_pool(name="cpool", bufs=2 * bpc) as cpool,
            tc.tile_pool(name="psum", bufs=8, space="PSUM") as ppool,
        ):
            wt = wpool.tile([L, 3 * L], F32)
            nc.sync.dma_start(out=wt[:, :], in_=w[:, :])
            vt = wpool.tile([L, 2 * L], F32)
            nc.sync.dma_start(out=vt[:, :], in_=v[:, :])
            A0w = wt[:, 0 * L:1 * L]
            C1w = wt[:, 1 * L:2 * L]
            Cw = wt[:, 2 * L:3 * L]

            xt: dict = {}
            yt: dict = {}
            carry: dict = {}
            eng_i = 0
            for ch in range(nchunk):
                for b in range(bpc):
                    r0 = b * t_len + ch * chunk * L
                    xt[b] = xpool.tile([L, chunk * C], F32, tag="xt", name=f"xt_{ch}_{b}")
                    if ch == 0:
                        # split the first block out so matmuls start early
                        nc.sync.dma_start(out=xt[b][:, 0:C],
                                          in_=x[r0:r0 + L, :])
                        nc.sync.dma_start(
                            out=xt[b][:, C:].rearrange("p (n c) -> p n c",
                                                       c=C),
                            in_=x[r0 + L:r0 + chunk * L, :].rearrange(
                                "(n p) c -> p n c", p=L),
                        )
                    else:
                        nc.sync.dma_start(
                            out=xt[b][:, :].rearrange("p (n c) -> p n c",
                                                      c=C),
                            in_=x[r0:r0 + chunk * L, :].rearrange(
                                "(n p) c -> p n c", p=L),
                        )
                    yt[b] = ypool.tile([L, chunk * C], F32, tag="yt", name=f"yt_{ch}_{b}")
                for j in range(chunk):
                    r = ch * chunk + j   # block index within each sequence
                    ps = {}
                    for b in range(bpc):
                        ps[b] = ppool.tile([L, C], F32, tag="ps", name=f"ps_{ch}_{j}_{b}")
                        cw = A0w if r == 0 else (C1w if r == 1 else Cw)
                        nc.tensor.matmul(ps[b][:, :], cw,
                                         xt[b][:, j * C:(j + 1) * C],
                                         start=True, stop=(r == 0))
                    if r >= 1:
                        vcol = slice(0, L) if r == 1 else slice(L, 2 * L)
                        for b in range(bpc):
                            p0 = 32 * b
                            nc.tensor.matmul(
                                ps[b][:, :],
                                vt[p0:p0 + 1, vcol],
                                carry[b][p0:p0 + 1, :],
                                start=False, stop=True,
                                tile_position=(96, 0) if p0 == 96 else None,
                            )
                    for b in range(bpc):
                        dst = yt[b][:, j * C:(j + 1) * C]
                        if eng_i % 2 == 0:
                            nc.vector.tensor_copy(out=dst, in_=ps[b][:, :])
                        else:
                            nc.scalar.copy(dst, ps[b][:, :])
                        eng_i += 1
                        if r < nblk - 1:
                            # carry = (debiased) last row of this block; the
                            # next block's v undoes any debias fold.
                            nct = cpool.tile([L, C], F32, tag="carry", name=f"carry_{ch}_{j}_{b}")
                            nc.scalar.dma_start(
                                out=nct[32 * b:32 * b + 1, :],
                                in_=yt[b][L - 1:L, j * C:(j + 1) * C],
                            )
                            carry[b] = nct
                for b in range(bpc):
                    r0 = b * t_len + ch * chunk * L
                    nc.gpsimd.dma_start(
                        out=y[r0:r0 + chunk * L, :].rearrange(
                            "(n p) c -> p n c", p=L),
                        in_=yt[b][:, :].rearrange("p (n c) -> p n c", c=C),
                    )
    nc.compile()
    return nc


_CACHE: dict = {}


def _get_program():
    if "nc" not in _CACHE:
        _CACHE["nc"] = build_program()
        _CACHE["w"] = _build_weights()
        _CACHE["v"] = _build_v()
    return _CACHE["nc"], _CACHE["w"], _CACHE["v"]


def _run(x: np.ndarray, trace: bool = False):
    nc, w, v = _get_program()
    in_maps = [
        {
            "x": np.ascontiguousarray(
                x[k * BPC:(k + 1) * BPC].reshape(BPC * T, C)),
            "w": w,
            "v": v,
        }
        for k in range(NCORES)
    ]
    res = bass_utils.run_bass_kernel_spmd(
        nc, in_maps, core_ids=list(range(NCORES)), trace=trace)
    y = np.concatenate(
        [r["y"].reshape(BPC, T, C) for r in res.results], axis=0)
    return y, res


def kernel(x) -> np.ndarray:
    x = np.asarray(x, dtype=np.float32)
    assert x.shape == (B, T, C), x.shape
    y, _ = _run(x, trace=False)
    return y
